# revision 20
# baseline (speedup 1.0000x reference)
"""Trainium2 Bass kernel for a diagonal SSM layer.

Reference computation (per batch row b, seq t):
    a_t = sigmoid(Wa @ x_t + bias)        [state=256]
    b_t = B @ x_t                         [state=256]
    h_t = a_t * h_{t-1} + b_t             (linear scan over t)
    y_t = C @ h_t + D @ x_t               [d_model=1024]

Distribution: data-parallel over batch (8 rows -> 8 NeuronCores),
weights replicated. Host pre-transposes and pre-quantizes the streams:
 - a-matmul runs fully in fp8(e4m3) DoubleRow mode (2 k-slabs per
   instruction); the sigmoid's flat slope at logit ~2.2 makes the
   quantization error negligible.
 - D-matmul contracts half its K channels in fp8 DoubleRow and half in
   bf16 -- the error budget (rel 2e-2 vs the f32 reference) allows fp8
   on only ~half of the dominant D@x term.
 - b-matmul and C-matmul stay bf16; b's error is amplified ~1.3x by the
   scan so it cannot afford fp8.
 - fp8 operands are exponent-shifted (x*2^-3, weights*2^3) so the
   uniform-distributed weights clear the e4m3 subnormal cutoff.
The scan runs along the SBUF free dimension via the hardware
TensorTensorScan instruction.
"""

import sys
import types

sys.path.insert(0, "/opt/trn_rl_repo")


def _ensure_axon_hooks_shim():
    # Some images lack antenv.axon_hooks; concourse imports it
    # unconditionally when BASS_TRACE is set. Provide a no-op shim so
    # tracing degrades gracefully instead of crashing.
    try:
        import antenv.axon_hooks  # noqa: F401
        return
    except ImportError:
        pass
    import antenv

    mod = types.ModuleType("antenv.axon_hooks")
    mod._hook = None

    def get_axon_ntff_profile_hook():
        return mod._hook

    def set_axon_ntff_profile_hook(hook):
        mod._hook = hook

    mod.get_axon_ntff_profile_hook = get_axon_ntff_profile_hook
    mod.set_axon_ntff_profile_hook = set_axon_ntff_profile_hook
    sys.modules["antenv.axon_hooks"] = mod
    antenv.axon_hooks = mod


_ensure_axon_hooks_shim()

from contextlib import ExitStack

import numpy as np

from concourse import bacc, bass, mybir, tile
from concourse.bass_utils import run_bass_kernel_spmd

D_MODEL = 1024
STATE = 256
SEQ = 4096
BATCH = 8
N_CORES = 8
P = 128

KD = D_MODEL // P  # 8 k-slabs over d_model
KS = STATE // P  # 2 slabs over state
K8 = 6  # k-slabs of the D contraction done in fp8 DoubleRow (must be even)
KDB = KD - K8  # bf16 k-slabs for D
K8OFF = 2  # first fp8 k-slab (slabs K8OFF..K8OFF+K8-1 are fp8, rest bf16)
SD = 2  # fp8 exponent shift: x*2^-SD, weights*2^SD
CHUNKS = [512] * 8
STARTS = [sum(CHUNKS[:i]) for i in range(len(CHUNKS))]
NCH = len(CHUNKS)

f32 = mybir.dt.float32
bf16 = mybir.dt.bfloat16
fp8 = mybir.dt.float8e4
ts = bass.ts
AF = mybir.ActivationFunctionType
ALU = mybir.AluOpType
DR = mybir.MatmulPerfMode.DoubleRow


def _build_nc():
    nc = bacc.Bacc("TRN2", target_bir_lowering=False, debug=False)

    # All inputs are pre-permuted by the host into the exact SBUF layout
    # (partition-major), so every load is a contiguous multi-KB line per
    # partition: large DMA descriptors deliver ~4x faster than the
    # 256-512B ones a device-side (k p)->(p k) rearrange would produce.
    x8 = nc.dram_tensor("x8", [P, NCH, KD, 512], fp8, kind="ExternalInput").ap()
    xbf = nc.dram_tensor("xbf", [P, NCH, KD, 512], bf16, kind="ExternalInput").ap()
    waT8 = nc.dram_tensor("waT8", [P, KD, STATE], fp8, kind="ExternalInput").ap()
    bT = nc.dram_tensor("bT", [P, KD, STATE], bf16, kind="ExternalInput").ap()
    cT = nc.dram_tensor("cT", [P, KS, D_MODEL], bf16, kind="ExternalInput").ap()
    dT8 = nc.dram_tensor("dT8", [P, K8, D_MODEL], fp8, kind="ExternalInput").ap()
    dTbf = nc.dram_tensor("dTbf", [P, KDB, D_MODEL], bf16, kind="ExternalInput").ap()
    bias = nc.dram_tensor("bias", [P, KS], f32, kind="ExternalInput").ap()
    y = nc.dram_tensor("y", [SEQ, D_MODEL], f32, kind="ExternalOutput").ap()

    with tile.TileContext(nc) as tc, ExitStack() as ctx:
        wpool = ctx.enter_context(tc.tile_pool(name="w", bufs=1))
        xpool = ctx.enter_context(tc.tile_pool(name="x", bufs=4))
        apool = ctx.enter_context(tc.tile_pool(name="a", bufs=2))
        hpool = ctx.enter_context(tc.tile_pool(name="h", bufs=2))
        ypool = ctx.enter_context(tc.tile_pool(name="yo", bufs=2))
        hbfpool = ctx.enter_context(tc.tile_pool(name="hbf", bufs=2))
        pa = ctx.enter_context(tc.tile_pool(name="pa", bufs=1, space="PSUM"))
        pb = ctx.enter_context(tc.tile_pool(name="pb", bufs=1, space="PSUM"))
        py = ctx.enter_context(tc.tile_pool(name="py", bufs=4, space="PSUM"))

        # Replicated weights, resident in SBUF for the whole kernel.
        # Emission order on the sync queue approximates earliest-deadline-
        # first: waT8 + x8[0] gate the very first matmul, bT/xbf[0] the b
        # phase, cT/dT* only the (pipelined, one chunk behind) y-phase.
        # bias rides the parallel SWDGE queue.
        waT8_sb = wpool.tile([P, KD, STATE], fp8)
        bT_sb = wpool.tile([P, KD, STATE], bf16)
        cT_sb = wpool.tile([P, KS, D_MODEL], bf16)
        dT8_sb = wpool.tile([P, K8, D_MODEL], fp8)
        dTbf_sb = wpool.tile([P, KDB, D_MODEL], bf16)
        bias_sb = wpool.tile([P, KS], f32)
        nc.gpsimd.dma_start(bias_sb[:], bias[:])

        x8_tiles = []
        xbf_tiles = []

        # Queued transfers on one DMA queue round-robin share its ~230GB/s,
        # so whatever gates the first matmuls must head its queue (only
        # sync/scalar/gpsimd can issue DMAs):
        #   sync:   waT8 then the x8 stream (a-gate) + y stores
        #   scalar: bT then the xbf stream (b-gate), later y-phase weights
        #   gpsimd: bias only (SWDGE, slow for bulk)
        def prefetch_xs(c):
            cs = CHUNKS[c]
            t8 = xpool.tile([P, KD, cs], fp8, tag="x8")
            nc.sync.dma_start(t8[:], x8[:, c])
            x8_tiles.append(t8)
            tb = xpool.tile([P, KD, cs], bf16, tag="xbf")
            nc.scalar.dma_start(tb[:], xbf[:, c])
            xbf_tiles.append(tb)

        nc.sync.dma_start(waT8_sb[:], waT8[:])
        nc.scalar.dma_start(bT_sb[:], bT[:])
        prefetch_xs(0)

        h_tiles = {}
        hbf_tiles = {}

        def emit_ab(c):
            cs = CHUNKS[c]
            x8t = x8_tiles[c]
            xbt = xbf_tiles[c]
            a_ps = pa.tile([P, KS, cs], f32, tag="a_ps")
            b_ps = pb.tile([P, KS, cs], f32, tag="b_ps")
            a_sb = apool.tile([P, KS, cs], f32, tag="a_sb")
            h_sb = hpool.tile([P, KS, cs], f32, tag="h_sb")
            prev_h = h_tiles.get(c - 1)
            h_bf = hbfpool.tile([P, KS, cs], bf16, tag="h_bf")
            # Both a s-groups back-to-back: a single fp8/bf16 PE mode
            # transition per chunk (each entry costs ~190ns of pipeline
            # refill on HW).
            for s in range(KS):
                for kp in range(KD // 2):
                    nc.tensor.matmul(
                        a_ps[:, s, :],
                        waT8_sb[:, 2 * kp : 2 * kp + 2, ts(s, P)],
                        x8t[:, 2 * kp : 2 * kp + 2, :],
                        start=(kp == 0),
                        stop=(kp == KD // 2 - 1),
                        perf_mode=DR,
                    )
            for s in range(KS):
                nc.scalar.activation(
                    a_sb[:, s, :], a_ps[:, s, :], AF.Sigmoid,
                    bias=bias_sb[:, s : s + 1],
                )
                for k in range(KD):
                    nc.tensor.matmul(
                        b_ps[:, s, :],
                        bT_sb[:, k, ts(s, P)],
                        xbt[:, k, :],
                        start=(k == 0),
                        stop=(k == KD - 1),
                    )
                init = 0.0 if prev_h is None else prev_h[:, s, CHUNKS[c - 1] - 1 : CHUNKS[c - 1]]
                nc.vector.tensor_tensor_scan(
                    h_sb[:, s, :], a_sb[:, s, :], b_ps[:, s, :], init,
                    op0=ALU.mult, op1=ALU.add,
                )
                nc.vector.tensor_copy(h_bf[:, s, :], h_sb[:, s, :])
            h_tiles[c] = h_sb
            hbf_tiles[c] = h_bf

        def emit_y(c, last=False):
            tt = CHUNKS[c] // P
            row0 = STARTS[c] // P
            x8t = x8_tiles[c]
            xbt = xbf_tiles[c]
            h_bf = hbf_tiles[c]
            y_sb = ypool.tile([P, tt, D_MODEL], f32, tag="y_sb")

            def mm_c(y_ps, t, n, first):
                for s in range(KS):
                    nc.tensor.matmul(
                        y_ps[:],
                        h_bf[:, s, ts(t, P)],
                        cT_sb[:, s, ts(n, 512)],
                        start=(first and s == 0),
                        stop=False,
                    )

            def mm_dbf(y_ps, t, n):
                for k in range(KDB):
                    nc.tensor.matmul(
                        y_ps[:],
                        xbt[:, (K8OFF + K8 + k) % KD, ts(t, P)],
                        dTbf_sb[:, k, ts(n, 512)],
                        start=False,
                        stop=False,
                    )

            def mm_d8(y_ps, t, n):
                for kp in range(K8 // 2):
                    nc.tensor.matmul(
                        y_ps[:],
                        x8t[:, K8OFF + 2 * kp : K8OFF + 2 * kp + 2, ts(t, P)],
                        dT8_sb[:, 2 * kp : 2 * kp + 2, ts(n, 512)],
                        start=False,
                        stop=(kp == K8 // 2 - 1),
                        perf_mode=DR,
                    )

            # Process t-blocks in pairs: all four bf16 (C + D-bf16) groups
            # of the pair first, then the four fp8 DoubleRow runs back to
            # back -- one fp8<->bf16 PE mode transition per pair (each
            # entry costs ~190ns of pipeline refill). The four open PSUM
            # tiles exactly fill the 4-buffer py pool.
            for tp in range(0, tt, 2):
                pair = range(tp, min(tp + 2, tt))
                tiles = {}
                for t in pair:
                    for n in range(2):
                        y_ps = py.tile([P, 512], f32)
                        tiles[t, n] = y_ps
                        mm_c(y_ps, t, n, True)
                        mm_dbf(y_ps, t, n)
                for t in pair:
                    for n in range(2):
                        mm_d8(tiles[t, n], t, n)
                for t in pair:
                    nc.vector.tensor_copy(y_sb[:, t, ts(0, 512)], tiles[t, 0][:])
                    nc.scalar.copy(y_sb[:, t, ts(1, 512)], tiles[t, 1][:])
                    if last:
                        # Tail trim: store each half as soon as its copy
                        # lands, alternating DMA queues.
                        nc.scalar.dma_start(
                            y[ts(row0 + t, P), ts(0, 512)], y_sb[:, t, ts(0, 512)]
                        )
                        nc.sync.dma_start(
                            y[ts(row0 + t, P), ts(1, 512)], y_sb[:, t, ts(1, 512)]
                        )
                    else:
                        nc.sync.dma_start(y[ts(row0 + t, P), :], y_sb[:, t, :])

        # Software pipeline: y-phase for chunk c runs while chunk c+1's
        # a/b matmuls fill the PE queue, hiding the sigmoid+scan latency
        # behind matmul work.
        for c in range(NCH):
            if c + 1 < NCH:
                prefetch_xs(c + 1)
            emit_ab(c)
            if c == 0:
                # y-phase weights: issued from the scalar program after
                # chunk 0's sigmoids, so their transfers don't round-robin
                # against the b-gate (xbf) on the same queue.
                nc.scalar.dma_start(cT_sb[:], cT[:])
                nc.scalar.dma_start(dTbf_sb[:], dTbf[:])
                nc.scalar.dma_start(dT8_sb[:], dT8[:])
            if c >= 1:
                emit_y(c - 1)
        emit_y(NCH - 1, last=True)

    nc.compile()
    return nc


_NC_CACHE = None
LAST_RESULTS = None


def kernel(x, Wa_w, Wa_b, B_w, C_w, D_w):
    global _NC_CACHE, LAST_RESULTS
    if _NC_CACHE is None:
        _NC_CACHE = _build_nc()
    nc = _NC_CACHE

    import ml_dtypes

    F8 = ml_dtypes.float8_e4m3fn
    BF = ml_dtypes.bfloat16
    up = float(2.0**SD)
    dn = float(2.0**-SD)

    def pkm(a, nk):
        # [nk*P, M] -> [P, nk, M]: the SBUF-resident layout, so the DMA
        # reads one contiguous multi-KB line per partition.
        return np.ascontiguousarray(a.reshape(nk, P, -1).transpose(1, 0, 2))

    x = np.asarray(x, dtype=np.float32)
    waT8 = pkm((np.asarray(Wa_w, np.float32).T * up).astype(F8), KD)
    bT = pkm(np.asarray(B_w, np.float32).T.astype(BF), KD)
    cT = pkm(np.asarray(C_w, np.float32).T.astype(BF), KS)
    dT = np.asarray(D_w, np.float32).T
    dT8 = pkm((dT[K8OFF * P : (K8OFF + K8) * P] * up).astype(F8), K8)
    dTbf = pkm(
        np.concatenate([dT[: K8OFF * P], dT[(K8OFF + K8) * P :]]).astype(BF), KDB
    )
    bias = np.ascontiguousarray(np.asarray(Wa_b, np.float32).reshape(KS, P).T)

    in_maps = []
    for i in range(N_CORES):
        xT = x[i].T  # [D_MODEL, SEQ]
        # [KD*P, NCH*512] -> [P, NCH, KD, 512]
        def pckt(a):
            return np.ascontiguousarray(
                a.reshape(KD, P, NCH, 512).transpose(1, 2, 0, 3)
            )

        in_maps.append(
            {
                "x8": pckt((xT * dn).astype(F8)),
                "xbf": pckt(xT.astype(BF)),
                "waT8": waT8,
                "bT": bT,
                "cT": cT,
                "dT8": dT8,
                "dTbf": dTbf,
                "bias": bias,
            }
        )

    LAST_RESULTS = run_bass_kernel_spmd(nc, in_maps, core_ids=list(range(N_CORES)))
    return np.stack([r["y"] for r in LAST_RESULTS.results], axis=0)


# revision 23
# speedup vs baseline: 1.1450x; 1.1450x over previous
"""Trainium2 Bass kernel for a diagonal SSM layer.

Reference computation (per batch row b, seq t):
    a_t = sigmoid(Wa @ x_t + bias)        [state=256]
    b_t = B @ x_t                         [state=256]
    h_t = a_t * h_{t-1} + b_t             (linear scan over t)
    y_t = C @ h_t + D @ x_t               [d_model=1024]

Distribution: data-parallel over batch (8 rows -> 8 NeuronCores),
weights replicated. Host pre-transposes and pre-quantizes the streams:
 - a-matmul runs fully in fp8(e4m3) DoubleRow mode (2 k-slabs per
   instruction); the sigmoid's flat slope at logit ~2.2 makes the
   quantization error negligible.
 - D-matmul contracts half its K channels in fp8 DoubleRow and half in
   bf16 -- the error budget (rel 2e-2 vs the f32 reference) allows fp8
   on only ~half of the dominant D@x term.
 - b-matmul and C-matmul stay bf16; b's error is amplified ~1.3x by the
   scan so it cannot afford fp8.
 - fp8 operands are exponent-shifted (x*2^-3, weights*2^3) so the
   uniform-distributed weights clear the e4m3 subnormal cutoff.
The scan runs along the SBUF free dimension via the hardware
TensorTensorScan instruction.
"""

import sys
import types

sys.path.insert(0, "/opt/trn_rl_repo")


def _ensure_axon_hooks_shim():
    # Some images lack antenv.axon_hooks; concourse imports it
    # unconditionally when BASS_TRACE is set. Provide a no-op shim so
    # tracing degrades gracefully instead of crashing.
    try:
        import antenv.axon_hooks  # noqa: F401
        return
    except ImportError:
        pass
    import antenv

    mod = types.ModuleType("antenv.axon_hooks")
    mod._hook = None

    def get_axon_ntff_profile_hook():
        return mod._hook

    def set_axon_ntff_profile_hook(hook):
        mod._hook = hook

    mod.get_axon_ntff_profile_hook = get_axon_ntff_profile_hook
    mod.set_axon_ntff_profile_hook = set_axon_ntff_profile_hook
    sys.modules["antenv.axon_hooks"] = mod
    antenv.axon_hooks = mod


_ensure_axon_hooks_shim()

from contextlib import ExitStack

import numpy as np

from concourse import bacc, bass, mybir, tile
from concourse.bass_utils import run_bass_kernel_spmd

D_MODEL = 1024
STATE = 256
SEQ = 4096
BATCH = 8
N_CORES = 8
P = 128

KD = D_MODEL // P  # 8 k-slabs over d_model
KS = STATE // P  # 2 slabs over state
K8 = 6  # k-slabs of the D contraction done in fp8 DoubleRow (must be even)
KDB = KD - K8  # bf16 k-slabs for D
K8OFF = 2  # first fp8 k-slab (slabs K8OFF..K8OFF+K8-1 are fp8, rest bf16)
SD = 2  # fp8 exponent shift: x*2^-SD, weights*2^SD
CHUNKS = [512] * 8
STARTS = [sum(CHUNKS[:i]) for i in range(len(CHUNKS))]
NCH = len(CHUNKS)

f32 = mybir.dt.float32
bf16 = mybir.dt.bfloat16
fp8 = mybir.dt.float8e4
ts = bass.ts
AF = mybir.ActivationFunctionType
ALU = mybir.AluOpType
DR = mybir.MatmulPerfMode.DoubleRow


def _build_nc():
    nc = bacc.Bacc("TRN2", target_bir_lowering=False, debug=False)

    # All inputs are pre-permuted by the host into the exact SBUF layout
    # (partition-major), so every load is a contiguous multi-KB line per
    # partition: large DMA descriptors deliver ~4x faster than the
    # 256-512B ones a device-side (k p)->(p k) rearrange would produce.
    x8 = nc.dram_tensor("x8", [P, NCH, KD, 512], fp8, kind="ExternalInput").ap()
    xbf = nc.dram_tensor("xbf", [P, NCH, KD, 512], bf16, kind="ExternalInput").ap()
    waT8 = nc.dram_tensor("waT8", [P, KD, STATE], fp8, kind="ExternalInput").ap()
    bT = nc.dram_tensor("bT", [P, KD, STATE], bf16, kind="ExternalInput").ap()
    cT = nc.dram_tensor("cT", [P, KS, D_MODEL], bf16, kind="ExternalInput").ap()
    dT8 = nc.dram_tensor("dT8", [P, K8, D_MODEL], fp8, kind="ExternalInput").ap()
    dTbf = nc.dram_tensor("dTbf", [P, KDB, D_MODEL], bf16, kind="ExternalInput").ap()
    bias = nc.dram_tensor("bias", [P, KS], f32, kind="ExternalInput").ap()
    y = nc.dram_tensor("y", [SEQ, D_MODEL], f32, kind="ExternalOutput").ap()

    with tile.TileContext(nc) as tc, ExitStack() as ctx:
        wpool = ctx.enter_context(tc.tile_pool(name="w", bufs=1))
        xpool = ctx.enter_context(tc.tile_pool(name="x", bufs=4))
        apool = ctx.enter_context(tc.tile_pool(name="a", bufs=2))
        hpool = ctx.enter_context(tc.tile_pool(name="h", bufs=2))
        ypool = ctx.enter_context(tc.tile_pool(name="yo", bufs=2))
        hbfpool = ctx.enter_context(tc.tile_pool(name="hbf", bufs=2))
        pa = ctx.enter_context(tc.tile_pool(name="pa", bufs=1, space="PSUM"))
        pb = ctx.enter_context(tc.tile_pool(name="pb", bufs=1, space="PSUM"))
        py = ctx.enter_context(tc.tile_pool(name="py", bufs=4, space="PSUM"))

        # Replicated weights, resident in SBUF for the whole kernel.
        # Emission order on the sync queue approximates earliest-deadline-
        # first: waT8 + x8[0] gate the very first matmul, bT/xbf[0] the b
        # phase, cT/dT* only the (pipelined, one chunk behind) y-phase.
        # bias rides the parallel SWDGE queue.
        waT8_sb = wpool.tile([P, KD, STATE], fp8)
        bT_sb = wpool.tile([P, KD, STATE], bf16)
        cT_sb = wpool.tile([P, KS, D_MODEL], bf16)
        dT8_sb = wpool.tile([P, K8, D_MODEL], fp8)
        dTbf_sb = wpool.tile([P, KDB, D_MODEL], bf16)
        bias_sb = wpool.tile([P, KS], f32)
        nc.gpsimd.dma_start(bias_sb[:], bias[:])

        x8_tiles = []
        xbf_tiles = []

        # Concurrent transfers on one DMA queue share its bandwidth by
        # per-packet round robin, so a transfer split into N pieces gets N
        # shares. The tensors gating the first matmuls (waT8+x8[0] for a,
        # bT+xbf[0] for b) are split 4-ways on their queues; later weights
        # ride the otherwise-idle gpsimd SWDGE queue. Steady-state x rides
        # sync, y stores ride scalar (only sync/scalar/gpsimd issue DMAs).
        def prefetch_xs(c, split=False):
            cs = CHUNKS[c]
            t8 = xpool.tile([P, KD, cs], fp8, tag="x8")
            if split:
                for kp in range(KD // 2):
                    nc.sync.dma_start(
                        t8[:, 2 * kp : 2 * kp + 2, :], x8[:, c, 2 * kp : 2 * kp + 2]
                    )
            else:
                nc.sync.dma_start(t8[:], x8[:, c])
            x8_tiles.append(t8)
            tb = xpool.tile([P, KD, cs], bf16, tag="xbf")
            q = nc.scalar if split else nc.sync
            if split:
                for kp in range(KD // 2):
                    q.dma_start(
                        tb[:, 2 * kp : 2 * kp + 2, :], xbf[:, c, 2 * kp : 2 * kp + 2]
                    )
            else:
                q.dma_start(tb[:], xbf[:, c])
            xbf_tiles.append(tb)

        nc.sync.dma_start(waT8_sb[:], waT8[:])
        nc.scalar.dma_start(bT_sb[:], bT[:])
        prefetch_xs(0, split=True)
        nc.sync.dma_start(dT8_sb[:], dT8[:])
        nc.gpsimd.dma_start(cT_sb[:], cT[:])
        nc.gpsimd.dma_start(dTbf_sb[:], dTbf[:])
        prefetch_xs(1)

        h_tiles = {}
        hbf_tiles = {}

        def emit_ab(c):
            cs = CHUNKS[c]
            x8t = x8_tiles[c]
            xbt = xbf_tiles[c]
            a_ps = pa.tile([P, KS, cs], f32, tag="a_ps")
            b_ps = pb.tile([P, KS, cs], f32, tag="b_ps")
            a_sb = apool.tile([P, KS, cs], f32, tag="a_sb")
            h_sb = hpool.tile([P, KS, cs], f32, tag="h_sb")
            prev_h = h_tiles.get(c - 1)
            h_bf = hbfpool.tile([P, KS, cs], bf16, tag="h_bf")
            # Both a s-groups back-to-back: a single fp8/bf16 PE mode
            # transition per chunk (each entry costs ~190ns of pipeline
            # refill on HW).
            for s in range(KS):
                for kp in range(KD // 2):
                    nc.tensor.matmul(
                        a_ps[:, s, :],
                        waT8_sb[:, 2 * kp : 2 * kp + 2, ts(s, P)],
                        x8t[:, 2 * kp : 2 * kp + 2, :],
                        start=(kp == 0),
                        stop=(kp == KD // 2 - 1),
                        perf_mode=DR,
                    )
            for s in range(KS):
                nc.scalar.activation(
                    a_sb[:, s, :], a_ps[:, s, :], AF.Sigmoid,
                    bias=bias_sb[:, s : s + 1],
                )
                for k in range(KD):
                    nc.tensor.matmul(
                        b_ps[:, s, :],
                        bT_sb[:, k, ts(s, P)],
                        xbt[:, k, :],
                        start=(k == 0),
                        stop=(k == KD - 1),
                    )
                init = 0.0 if prev_h is None else prev_h[:, s, CHUNKS[c - 1] - 1 : CHUNKS[c - 1]]
                nc.vector.tensor_tensor_scan(
                    h_sb[:, s, :], a_sb[:, s, :], b_ps[:, s, :], init,
                    op0=ALU.mult, op1=ALU.add,
                )
                nc.vector.tensor_copy(h_bf[:, s, :], h_sb[:, s, :])
            h_tiles[c] = h_sb
            hbf_tiles[c] = h_bf

        def emit_y(c, last=False):
            tt = CHUNKS[c] // P
            row0 = STARTS[c] // P
            x8t = x8_tiles[c]
            xbt = xbf_tiles[c]
            h_bf = hbf_tiles[c]
            y_sb = ypool.tile([P, tt, D_MODEL], f32, tag="y_sb")

            def mm_c(y_ps, t, n, first):
                for s in range(KS):
                    nc.tensor.matmul(
                        y_ps[:],
                        h_bf[:, s, ts(t, P)],
                        cT_sb[:, s, ts(n, 512)],
                        start=(first and s == 0),
                        stop=False,
                    )

            def mm_dbf(y_ps, t, n):
                for k in range(KDB):
                    nc.tensor.matmul(
                        y_ps[:],
                        xbt[:, (K8OFF + K8 + k) % KD, ts(t, P)],
                        dTbf_sb[:, k, ts(n, 512)],
                        start=False,
                        stop=False,
                    )

            def mm_d8(y_ps, t, n):
                for kp in range(K8 // 2):
                    nc.tensor.matmul(
                        y_ps[:],
                        x8t[:, K8OFF + 2 * kp : K8OFF + 2 * kp + 2, ts(t, P)],
                        dT8_sb[:, 2 * kp : 2 * kp + 2, ts(n, 512)],
                        start=False,
                        stop=(kp == K8 // 2 - 1),
                        perf_mode=DR,
                    )

            # Process t-blocks in pairs: all four bf16 (C + D-bf16) groups
            # of the pair first, then the four fp8 DoubleRow runs back to
            # back -- one fp8<->bf16 PE mode transition per pair (each
            # entry costs ~190ns of pipeline refill). The four open PSUM
            # tiles exactly fill the 4-buffer py pool.
            for tp in range(0, tt, 2):
                pair = range(tp, min(tp + 2, tt))
                tiles = {}
                for t in pair:
                    for n in range(2):
                        y_ps = py.tile([P, 512], f32)
                        tiles[t, n] = y_ps
                        mm_c(y_ps, t, n, True)
                        mm_dbf(y_ps, t, n)
                for t in pair:
                    for n in range(2):
                        mm_d8(tiles[t, n], t, n)
                for t in pair:
                    nc.vector.tensor_copy(y_sb[:, t, ts(0, 512)], tiles[t, 0][:])
                    nc.scalar.copy(y_sb[:, t, ts(1, 512)], tiles[t, 1][:])
                    if last:
                        # Tail trim: store each half as soon as its copy
                        # lands, alternating DMA queues.
                        nc.scalar.dma_start(
                            y[ts(row0 + t, P), ts(0, 512)], y_sb[:, t, ts(0, 512)]
                        )
                        nc.sync.dma_start(
                            y[ts(row0 + t, P), ts(1, 512)], y_sb[:, t, ts(1, 512)]
                        )
                    else:
                        nc.scalar.dma_start(y[ts(row0 + t, P), :], y_sb[:, t, :])

        # Software pipeline: y-phase for chunk c runs while chunk c+1's
        # a/b matmuls fill the PE queue, hiding the sigmoid+scan latency
        # behind matmul work.
        for c in range(NCH):
            if c + 2 < NCH:
                prefetch_xs(c + 2)
            emit_ab(c)
            if c >= 1:
                emit_y(c - 1)
        emit_y(NCH - 1, last=True)

    nc.compile()
    return nc


_NC_CACHE = None
LAST_RESULTS = None


def kernel(x, Wa_w, Wa_b, B_w, C_w, D_w):
    global _NC_CACHE, LAST_RESULTS
    if _NC_CACHE is None:
        _NC_CACHE = _build_nc()
    nc = _NC_CACHE

    import ml_dtypes

    F8 = ml_dtypes.float8_e4m3fn
    BF = ml_dtypes.bfloat16
    up = float(2.0**SD)
    dn = float(2.0**-SD)

    def pkm(a, nk):
        # [nk*P, M] -> [P, nk, M]: the SBUF-resident layout, so the DMA
        # reads one contiguous multi-KB line per partition.
        return np.ascontiguousarray(a.reshape(nk, P, -1).transpose(1, 0, 2))

    x = np.asarray(x, dtype=np.float32)
    waT8 = pkm((np.asarray(Wa_w, np.float32).T * up).astype(F8), KD)
    bT = pkm(np.asarray(B_w, np.float32).T.astype(BF), KD)
    cT = pkm(np.asarray(C_w, np.float32).T.astype(BF), KS)
    dT = np.asarray(D_w, np.float32).T
    dT8 = pkm((dT[K8OFF * P : (K8OFF + K8) * P] * up).astype(F8), K8)
    dTbf = pkm(
        np.concatenate([dT[: K8OFF * P], dT[(K8OFF + K8) * P :]]).astype(BF), KDB
    )
    bias = np.ascontiguousarray(np.asarray(Wa_b, np.float32).reshape(KS, P).T)

    in_maps = []
    for i in range(N_CORES):
        xT = x[i].T  # [D_MODEL, SEQ]
        # [KD*P, NCH*512] -> [P, NCH, KD, 512]
        def pckt(a):
            return np.ascontiguousarray(
                a.reshape(KD, P, NCH, 512).transpose(1, 2, 0, 3)
            )

        in_maps.append(
            {
                "x8": pckt((xT * dn).astype(F8)),
                "xbf": pckt(xT.astype(BF)),
                "waT8": waT8,
                "bT": bT,
                "cT": cT,
                "dT8": dT8,
                "dTbf": dTbf,
                "bias": bias,
            }
        )

    LAST_RESULTS = run_bass_kernel_spmd(nc, in_maps, core_ids=list(range(N_CORES)))
    return np.stack([r["y"] for r in LAST_RESULTS.results], axis=0)


# revision 24
# speedup vs baseline: 1.1585x; 1.0118x over previous
"""Trainium2 Bass kernel for a diagonal SSM layer.

Reference computation (per batch row b, seq t):
    a_t = sigmoid(Wa @ x_t + bias)        [state=256]
    b_t = B @ x_t                         [state=256]
    h_t = a_t * h_{t-1} + b_t             (linear scan over t)
    y_t = C @ h_t + D @ x_t               [d_model=1024]

Distribution: data-parallel over batch (8 rows -> 8 NeuronCores),
weights replicated. Host pre-transposes and pre-quantizes the streams:
 - a-matmul runs fully in fp8(e4m3) DoubleRow mode (2 k-slabs per
   instruction, 2x the bf16 MAC rate); the sigmoid's flat slope at
   logit ~2.2 makes the quantization error negligible.
 - D-matmul contracts 6 of its 8 K-slabs in fp8 DoubleRow and 2 in
   bf16 -- the error budget (rel 2e-2 vs the f32 reference) allows fp8
   on only part of the dominant D@x term. The slab choice and the
   exponent shift were picked by an exact numpy simulation of the
   quantization error (which matches HW to ~4 digits).
 - b-matmul and C-matmul stay bf16; b's error is amplified ~1.3x by the
   scan so it cannot afford fp8.
 - fp8 operands are exponent-shifted (x*2^-2, weights*2^2) so the
   uniform-distributed weights clear the e4m3 subnormal cutoff.
The scan runs along the SBUF free dimension via the hardware
TensorTensorScan instruction. The PE pays ~190ns per bf16<->fp8 mode
transition, so DoubleRow matmuls are batched into long runs.
"""

import sys
import types

sys.path.insert(0, "/opt/trn_rl_repo")


def _ensure_axon_hooks_shim():
    # Some images lack antenv.axon_hooks; concourse imports it
    # unconditionally when BASS_TRACE is set. Provide a no-op shim so
    # tracing degrades gracefully instead of crashing.
    try:
        import antenv.axon_hooks  # noqa: F401
        return
    except ImportError:
        pass
    import antenv

    mod = types.ModuleType("antenv.axon_hooks")
    mod._hook = None

    def get_axon_ntff_profile_hook():
        return mod._hook

    def set_axon_ntff_profile_hook(hook):
        mod._hook = hook

    mod.get_axon_ntff_profile_hook = get_axon_ntff_profile_hook
    mod.set_axon_ntff_profile_hook = set_axon_ntff_profile_hook
    sys.modules["antenv.axon_hooks"] = mod
    antenv.axon_hooks = mod


_ensure_axon_hooks_shim()

from contextlib import ExitStack

import numpy as np

from concourse import bacc, bass, mybir, tile
from concourse.bass_utils import run_bass_kernel_spmd

D_MODEL = 1024
STATE = 256
SEQ = 4096
BATCH = 8
N_CORES = 8
P = 128

KD = D_MODEL // P  # 8 k-slabs over d_model
KS = STATE // P  # 2 slabs over state
K8 = 6  # k-slabs of the D contraction done in fp8 DoubleRow (must be even)
KDB = KD - K8  # bf16 k-slabs for D
K8OFF = 2  # first fp8 k-slab (slabs K8OFF..K8OFF+K8-1 are fp8, rest bf16)
SD = 2  # fp8 exponent shift: x*2^-SD, weights*2^SD
CHUNKS = [512] * 8
STARTS = [sum(CHUNKS[:i]) for i in range(len(CHUNKS))]
NCH = len(CHUNKS)

f32 = mybir.dt.float32
bf16 = mybir.dt.bfloat16
fp8 = mybir.dt.float8e4
ts = bass.ts
AF = mybir.ActivationFunctionType
ALU = mybir.AluOpType
DR = mybir.MatmulPerfMode.DoubleRow


def _build_nc():
    nc = bacc.Bacc("TRN2", target_bir_lowering=False, debug=False)

    x8 = nc.dram_tensor("x8", [D_MODEL, SEQ], fp8, kind="ExternalInput").ap()
    xbf = nc.dram_tensor("xbf", [D_MODEL, SEQ], bf16, kind="ExternalInput").ap()
    waT8 = nc.dram_tensor("waT8", [D_MODEL, STATE], fp8, kind="ExternalInput").ap()
    bT = nc.dram_tensor("bT", [D_MODEL, STATE], bf16, kind="ExternalInput").ap()
    cT = nc.dram_tensor("cT", [STATE, D_MODEL], bf16, kind="ExternalInput").ap()
    dT8 = nc.dram_tensor("dT8", [K8 * P, D_MODEL], fp8, kind="ExternalInput").ap()
    dTbf = nc.dram_tensor("dTbf", [KDB * P, D_MODEL], bf16, kind="ExternalInput").ap()
    bias = nc.dram_tensor("bias", [P, KS], f32, kind="ExternalInput").ap()
    y = nc.dram_tensor("y", [SEQ, D_MODEL], f32, kind="ExternalOutput").ap()

    with tile.TileContext(nc) as tc, ExitStack() as ctx:
        wpool = ctx.enter_context(tc.tile_pool(name="w", bufs=1))
        xpool = ctx.enter_context(tc.tile_pool(name="x", bufs=4))
        apool = ctx.enter_context(tc.tile_pool(name="a", bufs=2))
        hpool = ctx.enter_context(tc.tile_pool(name="h", bufs=2))
        ypool = ctx.enter_context(tc.tile_pool(name="yo", bufs=2))
        hbfpool = ctx.enter_context(tc.tile_pool(name="hbf", bufs=2))
        pa = ctx.enter_context(tc.tile_pool(name="pa", bufs=1, space="PSUM"))
        pb = ctx.enter_context(tc.tile_pool(name="pb", bufs=1, space="PSUM"))
        py = ctx.enter_context(tc.tile_pool(name="py", bufs=4, space="PSUM"))

        # Replicated weights, resident in SBUF for the whole kernel.
        # Emission order on the sync queue approximates earliest-deadline-
        # first: waT8 + x8[0] gate the very first matmul, bT/xbf[0] the b
        # phase, cT/dT* only the (pipelined, one chunk behind) y-phase.
        # bias rides the parallel SWDGE queue.
        waT8_sb = wpool.tile([P, KD, STATE], fp8)
        bT_sb = wpool.tile([P, KD, STATE], bf16)
        cT_sb = wpool.tile([P, KS, D_MODEL], bf16)
        dT8_sb = wpool.tile([P, K8, D_MODEL], fp8)
        dTbf_sb = wpool.tile([P, KDB, D_MODEL], bf16)
        bias_sb = wpool.tile([P, KS], f32)
        nc.gpsimd.dma_start(bias_sb[:], bias[:])

        x8_tiles = []
        xbf_tiles = []

        def prefetch_xs(c, split=False):
            cs = CHUNKS[c]
            t8 = xpool.tile([P, KD, cs], fp8, tag="x8")
            if split:
                # Per-pair DMAs give the a-gate tensors more round-robin
                # shares of the queue, so the first matmuls start sooner.
                for kp in range(KD // 2):
                    nc.sync.dma_start(
                        t8[:, 2 * kp : 2 * kp + 2, :],
                        x8[
                            2 * kp * P : (2 * kp + 2) * P,
                            STARTS[c] : STARTS[c] + cs,
                        ].rearrange("(k p) t -> p k t", p=P),
                    )
            else:
                nc.sync.dma_start(
                    t8[:],
                    x8[:, STARTS[c] : STARTS[c] + cs].rearrange("(k p) t -> p k t", p=P),
                )
            x8_tiles.append(t8)
            tb = xpool.tile([P, KD, cs], bf16, tag="xbf")
            # chunk 0's xbf rides the scalar queue so the b-gate isn't
            # serialized behind the a-gate on the sync queue.
            q = nc.scalar if split else nc.sync
            q.dma_start(
                tb[:],
                xbf[:, STARTS[c] : STARTS[c] + cs].rearrange("(k p) t -> p k t", p=P),
            )
            xbf_tiles.append(tb)

        nc.sync.dma_start(waT8_sb[:], waT8.rearrange("(k p) m -> p k m", p=P))
        prefetch_xs(0, split=True)
        nc.sync.dma_start(bT_sb[:], bT.rearrange("(k p) m -> p k m", p=P))
        # y-phase weights ride the (otherwise idle in the prologue) scalar
        # queue so the a/b gates on the sync queue aren't delayed by them.
        nc.scalar.dma_start(cT_sb[:], cT.rearrange("(k p) m -> p k m", p=P))
        nc.scalar.dma_start(dT8_sb[:], dT8.rearrange("(k p) m -> p k m", p=P))
        nc.scalar.dma_start(dTbf_sb[:], dTbf.rearrange("(k p) m -> p k m", p=P))
        prefetch_xs(1)

        h_tiles = {}
        hbf_tiles = {}

        def emit_ab(c):
            cs = CHUNKS[c]
            x8t = x8_tiles[c]
            xbt = xbf_tiles[c]
            a_ps = pa.tile([P, KS, cs], f32, tag="a_ps")
            b_ps = pb.tile([P, KS, cs], f32, tag="b_ps")
            a_sb = apool.tile([P, KS, cs], f32, tag="a_sb")
            h_sb = hpool.tile([P, KS, cs], f32, tag="h_sb")
            prev_h = h_tiles.get(c - 1)
            h_bf = hbfpool.tile([P, KS, cs], bf16, tag="h_bf")
            # Both a s-groups back-to-back: a single fp8/bf16 PE mode
            # transition per chunk.
            for s in range(KS):
                for kp in range(KD // 2):
                    nc.tensor.matmul(
                        a_ps[:, s, :],
                        waT8_sb[:, 2 * kp : 2 * kp + 2, ts(s, P)],
                        x8t[:, 2 * kp : 2 * kp + 2, :],
                        start=(kp == 0),
                        stop=(kp == KD // 2 - 1),
                        perf_mode=DR,
                    )
            for s in range(KS):
                nc.scalar.activation(
                    a_sb[:, s, :], a_ps[:, s, :], AF.Sigmoid,
                    bias=bias_sb[:, s : s + 1],
                )
                for k in range(KD):
                    nc.tensor.matmul(
                        b_ps[:, s, :],
                        bT_sb[:, k, ts(s, P)],
                        xbt[:, k, :],
                        start=(k == 0),
                        stop=(k == KD - 1),
                    )
                init = 0.0 if prev_h is None else prev_h[:, s, CHUNKS[c - 1] - 1 : CHUNKS[c - 1]]
                nc.vector.tensor_tensor_scan(
                    h_sb[:, s, :], a_sb[:, s, :], b_ps[:, s, :], init,
                    op0=ALU.mult, op1=ALU.add,
                )
                nc.vector.tensor_copy(h_bf[:, s, :], h_sb[:, s, :])
            h_tiles[c] = h_sb
            hbf_tiles[c] = h_bf

        def emit_y(c, last=False):
            tt = CHUNKS[c] // P
            row0 = STARTS[c] // P
            x8t = x8_tiles[c]
            xbt = xbf_tiles[c]
            h_bf = hbf_tiles[c]
            y_sb = ypool.tile([P, tt, D_MODEL], f32, tag="y_sb")

            def mm_c(y_ps, t, n, first):
                for s in range(KS):
                    nc.tensor.matmul(
                        y_ps[:],
                        h_bf[:, s, ts(t, P)],
                        cT_sb[:, s, ts(n, 512)],
                        start=(first and s == 0),
                        stop=False,
                    )

            def mm_dbf(y_ps, t, n):
                for k in range(KDB):
                    nc.tensor.matmul(
                        y_ps[:],
                        xbt[:, (K8OFF + K8 + k) % KD, ts(t, P)],
                        dTbf_sb[:, k, ts(n, 512)],
                        start=False,
                        stop=False,
                    )

            def mm_d8(y_ps, t, n):
                for kp in range(K8 // 2):
                    nc.tensor.matmul(
                        y_ps[:],
                        x8t[:, K8OFF + 2 * kp : K8OFF + 2 * kp + 2, ts(t, P)],
                        dT8_sb[:, 2 * kp : 2 * kp + 2, ts(n, 512)],
                        start=False,
                        stop=(kp == K8 // 2 - 1),
                        perf_mode=DR,
                    )

            # Process t-blocks in pairs: all four bf16 (C + D-bf16) groups
            # of the pair first, then the four fp8 DoubleRow runs back to
            # back -- one fp8<->bf16 PE mode transition per pair. The four
            # open PSUM tiles exactly fill the 4-buffer py pool.
            for tp in range(0, tt, 2):
                pair = range(tp, min(tp + 2, tt))
                tiles = {}
                for t in pair:
                    for n in range(2):
                        y_ps = py.tile([P, 512], f32)
                        tiles[t, n] = y_ps
                        mm_c(y_ps, t, n, True)
                        mm_dbf(y_ps, t, n)
                for t in pair:
                    for n in range(2):
                        mm_d8(tiles[t, n], t, n)
                for t in pair:
                    nc.vector.tensor_copy(y_sb[:, t, ts(0, 512)], tiles[t, 0][:])
                    nc.scalar.copy(y_sb[:, t, ts(1, 512)], tiles[t, 1][:])
                    if last:
                        # Tail trim: store each half as soon as its copy
                        # lands, alternating DMA queues.
                        nc.scalar.dma_start(
                            y[ts(row0 + t, P), ts(0, 512)], y_sb[:, t, ts(0, 512)]
                        )
                        nc.sync.dma_start(
                            y[ts(row0 + t, P), ts(1, 512)], y_sb[:, t, ts(1, 512)]
                        )
                    else:
                        nc.scalar.dma_start(y[ts(row0 + t, P), :], y_sb[:, t, :])

        # Software pipeline: y-phase for chunk c runs while chunk c+1's
        # a/b matmuls fill the PE queue, hiding the sigmoid+scan latency
        # behind matmul work.
        for c in range(NCH):
            if c + 2 < NCH:
                prefetch_xs(c + 2)
            emit_ab(c)
            if c >= 1:
                emit_y(c - 1)
        emit_y(NCH - 1, last=True)

    nc.compile()
    return nc


_NC_CACHE = None
LAST_RESULTS = None


def kernel(x, Wa_w, Wa_b, B_w, C_w, D_w):
    global _NC_CACHE, LAST_RESULTS
    if _NC_CACHE is None:
        _NC_CACHE = _build_nc()
    nc = _NC_CACHE

    import ml_dtypes

    F8 = ml_dtypes.float8_e4m3fn
    BF = ml_dtypes.bfloat16
    up = float(2.0**SD)
    dn = float(2.0**-SD)

    x = np.asarray(x, dtype=np.float32)
    waT8 = (np.ascontiguousarray(np.asarray(Wa_w, np.float32).T) * up).astype(F8)
    bT = np.ascontiguousarray(np.asarray(B_w, np.float32).T).astype(BF)
    cT = np.ascontiguousarray(np.asarray(C_w, np.float32).T).astype(BF)
    dT = np.ascontiguousarray(np.asarray(D_w, np.float32).T)
    dT8 = (dT[K8OFF * P : (K8OFF + K8) * P] * up).astype(F8)
    dTbf = np.ascontiguousarray(
        np.concatenate([dT[: K8OFF * P], dT[(K8OFF + K8) * P :]])
    ).astype(BF)
    bias = np.ascontiguousarray(np.asarray(Wa_b, np.float32).reshape(KS, P).T)

    in_maps = []
    for i in range(N_CORES):
        xT = np.ascontiguousarray(x[i].T)
        in_maps.append(
            {
                "x8": (xT * dn).astype(F8),
                "xbf": xT.astype(BF),
                "waT8": waT8,
                "bT": bT,
                "cT": cT,
                "dT8": dT8,
                "dTbf": dTbf,
                "bias": bias,
            }
        )

    LAST_RESULTS = run_bass_kernel_spmd(nc, in_maps, core_ids=list(range(N_CORES)))
    return np.stack([r["y"] for r in LAST_RESULTS.results], axis=0)


# revision 25
# speedup vs baseline: 1.1625x; 1.0035x over previous
"""Trainium2 Bass kernel for a diagonal SSM layer.

Reference computation (per batch row b, seq t):
    a_t = sigmoid(Wa @ x_t + bias)        [state=256]
    b_t = B @ x_t                         [state=256]
    h_t = a_t * h_{t-1} + b_t             (linear scan over t)
    y_t = C @ h_t + D @ x_t               [d_model=1024]

Distribution: data-parallel over batch (8 rows -> 8 NeuronCores),
weights replicated. Host pre-transposes and pre-quantizes the streams:
 - a-matmul runs fully in fp8(e4m3) DoubleRow mode (2 k-slabs per
   instruction, 2x the bf16 MAC rate); the sigmoid's flat slope at
   logit ~2.2 makes the quantization error negligible.
 - D-matmul contracts 6 of its 8 K-slabs in fp8 DoubleRow and 2 in
   bf16 -- the error budget (rel 2e-2 vs the f32 reference) allows fp8
   on only part of the dominant D@x term. The slab choice and the
   exponent shift were picked by an exact numpy simulation of the
   quantization error (which matches HW to ~4 digits).
 - b-matmul and C-matmul stay bf16; b's error is amplified ~1.3x by the
   scan so it cannot afford fp8.
 - fp8 operands are exponent-shifted (x*2^-2, weights*2^2) so the
   uniform-distributed weights clear the e4m3 subnormal cutoff.
The scan runs along the SBUF free dimension via the hardware
TensorTensorScan instruction. The PE pays ~190ns per bf16<->fp8 mode
transition, so DoubleRow matmuls are batched into long runs.
"""

import sys
import types

sys.path.insert(0, "/opt/trn_rl_repo")


def _ensure_axon_hooks_shim():
    # Some images lack antenv.axon_hooks; concourse imports it
    # unconditionally when BASS_TRACE is set. Provide a no-op shim so
    # tracing degrades gracefully instead of crashing.
    try:
        import antenv.axon_hooks  # noqa: F401
        return
    except ImportError:
        pass
    import antenv

    mod = types.ModuleType("antenv.axon_hooks")
    mod._hook = None

    def get_axon_ntff_profile_hook():
        return mod._hook

    def set_axon_ntff_profile_hook(hook):
        mod._hook = hook

    mod.get_axon_ntff_profile_hook = get_axon_ntff_profile_hook
    mod.set_axon_ntff_profile_hook = set_axon_ntff_profile_hook
    sys.modules["antenv.axon_hooks"] = mod
    antenv.axon_hooks = mod


_ensure_axon_hooks_shim()

from contextlib import ExitStack

import numpy as np

from concourse import bacc, bass, mybir, tile
from concourse.bass_utils import run_bass_kernel_spmd

D_MODEL = 1024
STATE = 256
SEQ = 4096
BATCH = 8
N_CORES = 8
P = 128

KD = D_MODEL // P  # 8 k-slabs over d_model
KS = STATE // P  # 2 slabs over state
K8 = 6  # k-slabs of the D contraction done in fp8 DoubleRow (must be even)
KDB = KD - K8  # bf16 k-slabs for D
K8OFF = 2  # first fp8 k-slab (slabs K8OFF..K8OFF+K8-1 are fp8, rest bf16)
SD = 2  # fp8 exponent shift: x*2^-SD, weights*2^SD
CHUNKS = [512] * 8
STARTS = [sum(CHUNKS[:i]) for i in range(len(CHUNKS))]
NCH = len(CHUNKS)

f32 = mybir.dt.float32
bf16 = mybir.dt.bfloat16
fp8 = mybir.dt.float8e4
ts = bass.ts
AF = mybir.ActivationFunctionType
ALU = mybir.AluOpType
DR = mybir.MatmulPerfMode.DoubleRow


def _build_nc():
    nc = bacc.Bacc("TRN2", target_bir_lowering=False, debug=False)

    x8 = nc.dram_tensor("x8", [D_MODEL, SEQ], fp8, kind="ExternalInput").ap()
    xbf = nc.dram_tensor("xbf", [D_MODEL, SEQ], bf16, kind="ExternalInput").ap()
    waT8 = nc.dram_tensor("waT8", [D_MODEL, STATE], fp8, kind="ExternalInput").ap()
    bT = nc.dram_tensor("bT", [D_MODEL, STATE], bf16, kind="ExternalInput").ap()
    cT = nc.dram_tensor("cT", [STATE, D_MODEL], bf16, kind="ExternalInput").ap()
    dT8 = nc.dram_tensor("dT8", [K8 * P, D_MODEL], fp8, kind="ExternalInput").ap()
    dTbf = nc.dram_tensor("dTbf", [KDB * P, D_MODEL], bf16, kind="ExternalInput").ap()
    bias = nc.dram_tensor("bias", [P, KS], f32, kind="ExternalInput").ap()
    y = nc.dram_tensor("y", [SEQ, D_MODEL], f32, kind="ExternalOutput").ap()

    with tile.TileContext(nc) as tc, ExitStack() as ctx:
        wpool = ctx.enter_context(tc.tile_pool(name="w", bufs=1))
        xpool = ctx.enter_context(tc.tile_pool(name="x", bufs=4))
        apool = ctx.enter_context(tc.tile_pool(name="a", bufs=2))
        hpool = ctx.enter_context(tc.tile_pool(name="h", bufs=2))
        ypool = ctx.enter_context(tc.tile_pool(name="yo", bufs=2))
        hbfpool = ctx.enter_context(tc.tile_pool(name="hbf", bufs=2))
        pa = ctx.enter_context(tc.tile_pool(name="pa", bufs=1, space="PSUM"))
        pb = ctx.enter_context(tc.tile_pool(name="pb", bufs=1, space="PSUM"))
        py = ctx.enter_context(tc.tile_pool(name="py", bufs=4, space="PSUM"))

        # Replicated weights, resident in SBUF for the whole kernel.
        # Emission order on the sync queue approximates earliest-deadline-
        # first: waT8 + x8[0] gate the very first matmul, bT/xbf[0] the b
        # phase, cT/dT* only the (pipelined, one chunk behind) y-phase.
        # bias rides the parallel SWDGE queue.
        waT8_sb = wpool.tile([P, KD, STATE], fp8)
        bT_sb = wpool.tile([P, KD, STATE], bf16)
        cT_sb = wpool.tile([P, KS, D_MODEL], bf16)
        dT8_sb = wpool.tile([P, K8, D_MODEL], fp8)
        dTbf_sb = wpool.tile([P, KDB, D_MODEL], bf16)
        bias_sb = wpool.tile([P, KS], f32)
        nc.gpsimd.dma_start(bias_sb[:], bias[:])

        x8_tiles = []
        xbf_tiles = []

        def prefetch_xs(c, split=False):
            cs = CHUNKS[c]
            t8 = xpool.tile([P, KD, cs], fp8, tag="x8")
            if split:
                # Per-pair DMAs give the a-gate tensors more round-robin
                # shares of the queue, so the first matmuls start sooner.
                for kp in range(KD // 2):
                    nc.sync.dma_start(
                        t8[:, 2 * kp : 2 * kp + 2, :],
                        x8[
                            2 * kp * P : (2 * kp + 2) * P,
                            STARTS[c] : STARTS[c] + cs,
                        ].rearrange("(k p) t -> p k t", p=P),
                    )
            else:
                nc.sync.dma_start(
                    t8[:],
                    x8[:, STARTS[c] : STARTS[c] + cs].rearrange("(k p) t -> p k t", p=P),
                )
            x8_tiles.append(t8)
            tb = xpool.tile([P, KD, cs], bf16, tag="xbf")
            # chunk 0's xbf rides the scalar queue so the b-gate isn't
            # serialized behind the a-gate on the sync queue.
            q = nc.scalar if split else nc.sync
            q.dma_start(
                tb[:],
                xbf[:, STARTS[c] : STARTS[c] + cs].rearrange("(k p) t -> p k t", p=P),
            )
            xbf_tiles.append(tb)

        nc.sync.dma_start(waT8_sb[:], waT8.rearrange("(k p) m -> p k m", p=P))
        prefetch_xs(0, split=True)
        nc.sync.dma_start(bT_sb[:], bT.rearrange("(k p) m -> p k m", p=P))
        # y-phase weights ride the (otherwise idle in the prologue) scalar
        # queue so the a/b gates on the sync queue aren't delayed by them.
        nc.scalar.dma_start(cT_sb[:], cT.rearrange("(k p) m -> p k m", p=P))
        nc.scalar.dma_start(dT8_sb[:], dT8.rearrange("(k p) m -> p k m", p=P))
        nc.scalar.dma_start(dTbf_sb[:], dTbf.rearrange("(k p) m -> p k m", p=P))
        prefetch_xs(1)

        h_tiles = {}
        hbf_tiles = {}

        def emit_ab(c):
            cs = CHUNKS[c]
            x8t = x8_tiles[c]
            xbt = xbf_tiles[c]
            a_ps = pa.tile([P, KS, cs], f32, tag="a_ps")
            b_ps = pb.tile([P, KS, cs], f32, tag="b_ps")
            a_sb = apool.tile([P, KS, cs], f32, tag="a_sb")
            h_sb = hpool.tile([P, KS, cs], f32, tag="h_sb")
            prev_h = h_tiles.get(c - 1)
            h_bf = hbfpool.tile([P, KS, cs], bf16, tag="h_bf")
            # Both a s-groups back-to-back: a single fp8/bf16 PE mode
            # transition per chunk.
            for s in range(KS):
                for kp in range(KD // 2):
                    nc.tensor.matmul(
                        a_ps[:, s, :],
                        waT8_sb[:, 2 * kp : 2 * kp + 2, ts(s, P)],
                        x8t[:, 2 * kp : 2 * kp + 2, :],
                        start=(kp == 0),
                        stop=(kp == KD // 2 - 1),
                        perf_mode=DR,
                    )
            for s in range(KS):
                nc.scalar.activation(
                    a_sb[:, s, :], a_ps[:, s, :], AF.Sigmoid,
                    bias=bias_sb[:, s : s + 1],
                )
                for k in range(KD):
                    nc.tensor.matmul(
                        b_ps[:, s, :],
                        bT_sb[:, k, ts(s, P)],
                        xbt[:, k, :],
                        start=(k == 0),
                        stop=(k == KD - 1),
                    )
                init = 0.0 if prev_h is None else prev_h[:, s, CHUNKS[c - 1] - 1 : CHUNKS[c - 1]]
                nc.vector.tensor_tensor_scan(
                    h_sb[:, s, :], a_sb[:, s, :], b_ps[:, s, :], init,
                    op0=ALU.mult, op1=ALU.add,
                )
                nc.vector.tensor_copy(h_bf[:, s, :], h_sb[:, s, :])
            h_tiles[c] = h_sb
            hbf_tiles[c] = h_bf

        def emit_y(c, last=False):
            tt = CHUNKS[c] // P
            row0 = STARTS[c] // P
            x8t = x8_tiles[c]
            xbt = xbf_tiles[c]
            h_bf = hbf_tiles[c]
            y_sb = ypool.tile([P, tt, D_MODEL], f32, tag="y_sb")

            def mm_c(y_ps, t, n, first):
                for s in range(KS):
                    nc.tensor.matmul(
                        y_ps[:],
                        h_bf[:, s, ts(t, P)],
                        cT_sb[:, s, ts(n, 512)],
                        start=(first and s == 0),
                        stop=False,
                    )

            def mm_dbf(y_ps, t, n):
                for k in range(KDB):
                    nc.tensor.matmul(
                        y_ps[:],
                        xbt[:, (K8OFF + K8 + k) % KD, ts(t, P)],
                        dTbf_sb[:, k, ts(n, 512)],
                        start=False,
                        stop=False,
                    )

            def mm_d8(y_ps, t, n):
                for kp in range(K8 // 2):
                    nc.tensor.matmul(
                        y_ps[:],
                        x8t[:, K8OFF + 2 * kp : K8OFF + 2 * kp + 2, ts(t, P)],
                        dT8_sb[:, 2 * kp : 2 * kp + 2, ts(n, 512)],
                        start=False,
                        stop=(kp == K8 // 2 - 1),
                        perf_mode=DR,
                    )

            # Process one n-half of all four t-blocks at a time: the four
            # bf16 (C + D-bf16) groups first, then the four fp8 DoubleRow
            # runs back to back -- 3 fp8<->bf16 PE mode transitions per
            # chunk (the n=1 DR run lands adjacent to the next chunk's
            # DoubleRow a-matmuls). The four open PSUM tiles exactly fill
            # the 4-buffer py pool.
            for n in range(2):
                tiles = {}
                for t in range(tt):
                    y_ps = py.tile([P, 512], f32)
                    tiles[t] = y_ps
                    mm_c(y_ps, t, n, True)
                    mm_dbf(y_ps, t, n)
                for t in range(tt):
                    mm_d8(tiles[t], t, n)
                for t in range(tt):
                    q = nc.vector if (t % 2 == 0) else nc.scalar
                    if t % 2 == 0:
                        nc.vector.tensor_copy(y_sb[:, t, ts(n, 512)], tiles[t][:])
                    else:
                        nc.scalar.copy(y_sb[:, t, ts(n, 512)], tiles[t][:])
                    if last:
                        # Tail trim: store each half as soon as its copy
                        # lands, alternating DMA queues.
                        q = nc.scalar if n == 0 else nc.sync
                        q.dma_start(
                            y[ts(row0 + t, P), ts(n, 512)], y_sb[:, t, ts(n, 512)]
                        )
                    elif n == 1:
                        nc.scalar.dma_start(y[ts(row0 + t, P), :], y_sb[:, t, :])

        # Software pipeline: y-phase for chunk c runs while chunk c+1's
        # a/b matmuls fill the PE queue, hiding the sigmoid+scan latency
        # behind matmul work.
        for c in range(NCH):
            if c + 2 < NCH:
                prefetch_xs(c + 2)
            emit_ab(c)
            if c >= 1:
                emit_y(c - 1)
        emit_y(NCH - 1, last=True)

    nc.compile()
    return nc


_NC_CACHE = None
LAST_RESULTS = None


def kernel(x, Wa_w, Wa_b, B_w, C_w, D_w):
    global _NC_CACHE, LAST_RESULTS
    if _NC_CACHE is None:
        _NC_CACHE = _build_nc()
    nc = _NC_CACHE

    import ml_dtypes

    F8 = ml_dtypes.float8_e4m3fn
    BF = ml_dtypes.bfloat16
    up = float(2.0**SD)
    dn = float(2.0**-SD)

    x = np.asarray(x, dtype=np.float32)
    waT8 = (np.ascontiguousarray(np.asarray(Wa_w, np.float32).T) * up).astype(F8)
    bT = np.ascontiguousarray(np.asarray(B_w, np.float32).T).astype(BF)
    cT = np.ascontiguousarray(np.asarray(C_w, np.float32).T).astype(BF)
    dT = np.ascontiguousarray(np.asarray(D_w, np.float32).T)
    dT8 = (dT[K8OFF * P : (K8OFF + K8) * P] * up).astype(F8)
    dTbf = np.ascontiguousarray(
        np.concatenate([dT[: K8OFF * P], dT[(K8OFF + K8) * P :]])
    ).astype(BF)
    bias = np.ascontiguousarray(np.asarray(Wa_b, np.float32).reshape(KS, P).T)

    in_maps = []
    for i in range(N_CORES):
        xT = np.ascontiguousarray(x[i].T)
        in_maps.append(
            {
                "x8": (xT * dn).astype(F8),
                "xbf": xT.astype(BF),
                "waT8": waT8,
                "bT": bT,
                "cT": cT,
                "dT8": dT8,
                "dTbf": dTbf,
                "bias": bias,
            }
        )

    LAST_RESULTS = run_bass_kernel_spmd(nc, in_maps, core_ids=list(range(N_CORES)))
    return np.stack([r["y"] for r in LAST_RESULTS.results], axis=0)
